# revision 2
# baseline (speedup 1.0000x reference)
"""BertLinearSelfAttention on 8 Trainium2 NeuronCores.

Problem (per reference):
  q = hs @ Wq.T + bq ; k = hs @ Wk.T + bk ; v = hs @ Wv.T + bv   (B,S,D)
  per head: scores = q @ k.T ; probs = scores * (mask >= 0) ; ctx = probs @ v
  B=2, S=2048, D=1024, H=16, HD=64. No softmax, binary key mask.

Sharding: core c = 4*b + g handles batch b and head group g (4 heads,
256 output features). Each core runs the same SPMD program on its own
slice; output is gathered host-side.

Key algebraic move: (scores * mask_k) @ v == scores @ (mask_k * v) —
the binary key mask is applied to V rows (tiny) instead of the S x S
probs matrix (huge).

On-chip layout (per core):
  xT   [D, S]    hidden transposed via PE-identity transposes (exact)
  qT,kT [256, S] projection outputs kept transposed (feature-major)
  v    [S, 256]  natural layout, bias via K=1 ones-matmul, key mask
                 applied on the PSUM->SBUF copy
  scoresT tiles [s_k 128, s_q 512] = kT.T @ qT per head (K=64, two heads
                 packed in the PE array via disjoint row groups)
  ctxT [64, s_q] accumulated over s_k chunks (fp32r, M=64)
All matmuls run as float32r (TF32-like, ~1.5e-4 rel err, 1 cyc/row);
transposes are exact fp32.
"""
import numpy as np
import concourse.bass as bass
import concourse.mybir as mybir
import concourse.tile as tile
from concourse import bacc
from concourse.bass import ts
from concourse.bass_utils import run_bass_kernel_spmd
from concourse.masks import make_identity

f32 = mybir.dt.float32
f32r = mybir.dt.float32r
AF = mybir.ActivationFunctionType
OP = mybir.AluOpType

B = 2
S = 2048
D = 1024
DL = 256          # output features per core (4 heads x 64)
KC = D // 128     # 8 contraction chunks
SC = S // 128     # 16 sequence chunks
MC = DL // 128    # 2 feature chunks / head pairs
SQW = 512         # attention s_q strip width
NSQ = S // SQW    # 4 strips
N_CORES = 8

_cache = {}


def _build():
    nc = bacc.Bacc("TRN2", target_bir_lowering=False, debug=False,
                   num_devices=N_CORES)
    X = nc.declare_dram_parameter("x", [S, D], f32, isOutput=False)
    WQ = nc.declare_dram_parameter("wqt", [D, DL], f32r, isOutput=False)
    WK = nc.declare_dram_parameter("wkt", [D, DL], f32r, isOutput=False)
    WV = nc.declare_dram_parameter("wvt", [D, DL], f32r, isOutput=False)
    BQ = nc.declare_dram_parameter("bq2", [128, MC], f32, isOutput=False)
    BK = nc.declare_dram_parameter("bk2", [128, MC], f32, isOutput=False)
    BV = nc.declare_dram_parameter("bv", [1, DL], f32r, isOutput=False)
    ONE = nc.declare_dram_parameter("ones", [1, 128], f32r, isOutput=False)
    MSK = nc.declare_dram_parameter("mask", [S], f32, isOutput=False)
    OUT = nc.declare_dram_parameter("out", [DL, S], f32, isOutput=True)

    with tile.TileContext(nc) as tc:
        with tc.tile_pool(name="sb", bufs=1) as sb, \
             tc.tile_pool(name="ps", bufs=4, space="PSUM") as ps, \
             tc.tile_pool(name="psc", bufs=4, space="PSUM") as psc, \
             tc.tile_pool(name="stg", bufs=4) as stg:

            # ---- persistent operands -------------------------------------
            wqt = sb.tile([128, KC * DL], f32r, tag="wqt")
            nc.sync.dma_start(wqt[:].rearrange("p (c m) -> p c m", c=KC),
                              WQ.ap().rearrange("(c p) m -> p c m", p=128))
            wkt = sb.tile([128, KC * DL], f32r, tag="wkt")
            nc.sync.dma_start(wkt[:].rearrange("p (c m) -> p c m", c=KC),
                              WK.ap().rearrange("(c p) m -> p c m", p=128))
            wvt = sb.tile([128, KC * DL], f32r, tag="wvt")
            nc.sync.dma_start(wvt[:].rearrange("p (c m) -> p c m", c=KC),
                              WV.ap().rearrange("(c p) m -> p c m", p=128))
            bq2 = sb.tile([128, MC], f32, tag="bq2")
            nc.sync.dma_start(bq2[:], BQ[:, :])
            bk2 = sb.tile([128, MC], f32, tag="bk2")
            nc.sync.dma_start(bk2[:], BK[:, :])
            bv_t = sb.tile([1, DL], f32r, tag="bv")
            nc.sync.dma_start(bv_t[:], BV[:, :])
            ones_t = sb.tile([1, 128], f32r, tag="ones")
            nc.sync.dma_start(ones_t[:], ONE[:, :])
            maskT = sb.tile([128, SC], f32, tag="maskT")
            nc.sync.dma_start(maskT[:], MSK.ap().rearrange("(c p) -> p c", p=128))
            binm = sb.tile([128, SC], f32, tag="binm")
            nc.vector.tensor_scalar(binm[:], maskT[:], 0.0, None, OP.is_ge)
            ident = sb.tile([128, 128], f32, tag="ident")
            make_identity(nc, ident[:])

            qT = [sb.tile([128, S], f32r, tag=f"qT{m}", name=f"qT{m}") for m in range(MC)]
            kT = [sb.tile([128, S], f32r, tag=f"kT{m}", name=f"kT{m}") for m in range(MC)]
            v_sb = sb.tile([128, SC * DL], f32r, tag="v_sb")

            eng = 0  # DVE/ACT alternator for PSUM->SBUF drains

            # ---- phase A: transpose x, projections -----------------------
            with tc.tile_pool(name="xs", bufs=6) as xs, \
                 tc.tile_pool(name="xtp", bufs=1) as xtp:
                xt = [xtp.tile([128, S], f32r, tag=f"xt{k}", name=f"xt{k}") for k in range(KC)]
                for sb_i in range(S // SQW):
                    xch = []
                    for j in range(4):
                        sc = sb_i * 4 + j
                        xc = xs.tile([128, D], f32, tag="xc")
                        nc.sync.dma_start(xc[:], X[ts(sc, 128), :])
                        xch.append(xc)
                    # transpose this 512-row block of x into xT
                    for kc in range(KC):
                        pt = ps.tile([128, 512], f32, tag="sc")
                        for j in range(4):
                            nc.tensor.transpose(pt[:, ts(j, 128)],
                                                xch[j][:, ts(kc, 128)], ident[:])
                        if eng == 0:
                            nc.vector.tensor_copy(xt[kc][:, ts(sb_i, 512)], pt[:])
                        else:
                            nc.scalar.copy(xt[kc][:, ts(sb_i, 512)], pt[:])
                        eng ^= 1
                    # q/k projections for this s-block (outputs transposed)
                    for mc in range(MC):
                        for w_t, bias2, dst in ((wqt, bq2, qT), (wkt, bk2, kT)):
                            pt = ps.tile([128, 512], f32, tag="sc")
                            for kc in range(KC):
                                nc.tensor.matmul(
                                    pt[:],
                                    w_t[:, kc * DL + mc * 128:kc * DL + mc * 128 + 128],
                                    xt[kc][:, ts(sb_i, 512)],
                                    start=(kc == 0), stop=(kc == KC - 1))
                            if eng == 0:
                                nc.vector.tensor_scalar_add(
                                    dst[mc][:, ts(sb_i, 512)], pt[:],
                                    bias2[:, mc:mc + 1])
                            else:
                                nc.scalar.add(dst[mc][:, ts(sb_i, 512)], pt[:],
                                              bias2[:, mc:mc + 1])
                            eng ^= 1
                    # v projection in natural layout + bias + key mask
                    for j in range(4):
                        sc = sb_i * 4 + j
                        pv = ps.tile([128, 512], f32, tag="sc")
                        for kc in range(KC):
                            nc.tensor.matmul(pv[:, 0:DL],
                                             xt[kc][:, ts(sc, 128)],
                                             wvt[:, ts(kc, DL)],
                                             start=(kc == 0), stop=False)
                        nc.tensor.matmul(pv[:, 0:DL], ones_t[:], bv_t[:],
                                         start=False, stop=True)
                        if eng == 0:
                            nc.vector.tensor_scalar_mul(v_sb[:, ts(sc, DL)],
                                                        pv[:, 0:DL],
                                                        binm[:, sc:sc + 1])
                        else:
                            nc.scalar.activation(v_sb[:, ts(sc, DL)], pv[:, 0:DL],
                                                 AF.Copy, scale=binm[:, sc:sc + 1])
                        eng ^= 1

            # ---- phase B: attention --------------------------------------
            with tc.tile_pool(name="probs", bufs=40) as pp:
                for hp in range(MC):
                    for sq in range(NSQ):
                        pbs = []
                        for sk in range(SC):
                            for h in range(2):
                                spt = ps.tile([128, 512], f32, tag="sc")
                                nc.tensor.matmul(
                                    spt[:],
                                    kT[hp][ts(h, 64), ts(sk, 128)],
                                    qT[hp][ts(h, 64), ts(sq, SQW)],
                                    start=True, stop=True)
                                pb = pp.tile([128, SQW], f32r, tag="pb")
                                if eng == 0:
                                    nc.vector.tensor_copy(pb[:], spt[:])
                                else:
                                    nc.scalar.copy(pb[:], spt[:])
                                eng ^= 1
                                pbs.append(pb)
                        cts = [psc.tile([64, SQW], f32, tag="ctx", name=f"ct{hp}_{sq}_{i}") for i in range(2)]
                        for sk in range(SC):
                            for h in range(2):
                                nc.tensor.matmul(
                                    cts[h][:],
                                    v_sb[:, sk * DL + hp * 128 + h * 64:
                                         sk * DL + hp * 128 + h * 64 + 64],
                                    pbs[2 * sk + h][:],
                                    start=(sk == 0), stop=(sk == SC - 1))
                        stage = stg.tile([128, SQW], f32, tag="st")
                        nc.vector.tensor_copy(stage[0:64, :], cts[0][:])
                        nc.scalar.copy(stage[64:128, :], cts[1][:])
                        nc.sync.dma_start(
                            OUT[hp * 128:(hp + 1) * 128, ts(sq, SQW)], stage[:])

    nc.compile()
    return nc


def _get_nc():
    if "nc" not in _cache:
        _cache["nc"] = _build()
    return _cache["nc"]


def _make_in_maps(hidden_states, attention_mask, Wq, bq, Wk, bk, Wv, bv):
    hs = np.ascontiguousarray(np.asarray(hidden_states, dtype=np.float32))
    am = np.asarray(attention_mask, dtype=np.float32)
    ones = np.ones((1, 128), np.float32)
    in_maps = []
    for c in range(N_CORES):
        b, g = divmod(c, 4)
        sl = slice(g * DL, (g + 1) * DL)
        in_maps.append({
            "x": hs[b],
            "wqt": np.ascontiguousarray(np.asarray(Wq, np.float32)[sl, :].T),
            "wkt": np.ascontiguousarray(np.asarray(Wk, np.float32)[sl, :].T),
            "wvt": np.ascontiguousarray(np.asarray(Wv, np.float32)[sl, :].T),
            "bq2": np.ascontiguousarray(
                np.asarray(bq, np.float32)[sl].reshape(MC, 128).T),
            "bk2": np.ascontiguousarray(
                np.asarray(bk, np.float32)[sl].reshape(MC, 128).T),
            "bv": np.ascontiguousarray(
                np.asarray(bv, np.float32)[sl].reshape(1, DL)),
            "ones": ones,
            "mask": np.ascontiguousarray(am[b, 0, 0, :]),
        })
    return in_maps


def _gather(results):
    out = np.empty((B, S, D), np.float32)
    for c in range(N_CORES):
        b, g = divmod(c, 4)
        out[b, :, g * DL:(g + 1) * DL] = results[c]["out"].T
    return out


def run_sharded(in_maps, **kw):
    nc = _get_nc()
    return run_bass_kernel_spmd(nc, in_maps, core_ids=list(range(N_CORES)), **kw)


def kernel(hidden_states, attention_mask, Wq, bq, Wk, bk, Wv, bv):
    in_maps = _make_in_maps(hidden_states, attention_mask,
                            Wq, bq, Wk, bk, Wv, bv)
    res = run_sharded(in_maps)
    return _gather(res.results)


# revision 3
# speedup vs baseline: 1.0976x; 1.0976x over previous
"""BertLinearSelfAttention on 8 Trainium2 NeuronCores.

Problem (per reference):
  q = hs @ Wq.T + bq ; k = hs @ Wk.T + bk ; v = hs @ Wv.T + bv   (B,S,D)
  per head: scores = q @ k.T ; probs = scores * (mask >= 0) ; ctx = probs @ v
  B=2, S=2048, D=1024, H=16, HD=64. No softmax, binary key mask.

Sharding: core c = 4*b + g handles batch b and head group g (4 heads,
256 output features). Each core runs the same SPMD program on its own
slice; output is gathered host-side.

Key algebraic move: (scores * mask_k) @ v == scores @ (mask_k * v) —
the binary key mask is applied to V rows (tiny) instead of the S x S
probs matrix (huge).

On-chip layout (per core):
  xT   [D, S]    hidden transposed via PE-identity transposes (exact)
  qT,kT [256, S] projection outputs kept transposed (feature-major)
  v    [S, 256]  natural layout, bias via K=1 ones-matmul, key mask
                 applied on the PSUM->SBUF copy
  scoresT pair tiles [s_k 128, 2 x s_q 512] = kT.T @ qT for both heads of
                 a pair (K=64, packed in the PE array via disjoint row
                 groups, two PSUM banks), drained by one wide copy
  ctxT [64, s_q] accumulated over s_k chunks (fp32r, M=64)
All matmuls run as float32r (TF32-like, ~1.5e-4 rel err, 1 cyc/row);
transposes are exact fp32.
"""
import numpy as np
import concourse.bass as bass
import concourse.mybir as mybir
import concourse.tile as tile
from concourse import bacc
from concourse.bass import ts
from concourse.bass_utils import run_bass_kernel_spmd

f32 = mybir.dt.float32
f32r = mybir.dt.float32r
AF = mybir.ActivationFunctionType
OP = mybir.AluOpType

B = 2
S = 2048
D = 1024
DL = 256          # output features per core (4 heads x 64)
KC = D // 128     # 8 contraction chunks
SC = S // 128     # 16 sequence chunks
MC = DL // 128    # 2 feature chunks / head pairs
SQW = 512         # attention s_q strip width
NSQ = S // SQW    # 4 strips
N_CORES = 8

_cache = {}


def _build():
    nc = bacc.Bacc("TRN2", target_bir_lowering=False, debug=False,
                   num_devices=N_CORES)
    X = nc.declare_dram_parameter("x", [S, D], f32, isOutput=False)
    IDN = nc.declare_dram_parameter("idn", [128, 128], f32, isOutput=False)
    WQ = nc.declare_dram_parameter("wqt", [D, DL], f32r, isOutput=False)
    WK = nc.declare_dram_parameter("wkt", [D, DL], f32r, isOutput=False)
    WV = nc.declare_dram_parameter("wvt", [D, DL], f32r, isOutput=False)
    BQ = nc.declare_dram_parameter("bq2", [128, MC], f32, isOutput=False)
    BK = nc.declare_dram_parameter("bk2", [128, MC], f32, isOutput=False)
    BV = nc.declare_dram_parameter("bv", [1, DL], f32r, isOutput=False)
    ONE = nc.declare_dram_parameter("ones", [1, 128], f32r, isOutput=False)
    MSK = nc.declare_dram_parameter("mask", [S], f32, isOutput=False)
    OUT = nc.declare_dram_parameter("out", [DL, S], f32, isOutput=True)

    with tile.TileContext(nc) as tc:
        with tc.tile_pool(name="sb", bufs=1) as sb, \
             tc.tile_pool(name="stg", bufs=4) as stg:

            # ---- first wave of DMAs: what the PE needs soonest -----------
            ident = sb.tile([128, 128], f32, tag="ident")
            nc.sync.dma_start(ident[:], IDN[:, :])

            qT = [sb.tile([128, S], f32r, tag=f"qT{m}", name=f"qT{m}")
                  for m in range(MC)]
            kT = [sb.tile([128, S], f32r, tag=f"kT{m}", name=f"kT{m}")
                  for m in range(MC)]
            v_sb = sb.tile([128, SC * DL], f32r, tag="v_sb")

            eng = 0  # DVE/ACT alternator for PSUM->SBUF drains

            with tc.tile_pool(name="xs", bufs=6) as xs, \
                 tc.tile_pool(name="xtp", bufs=1) as xtp, \
                 tc.tile_pool(name="psA", bufs=6, space="PSUM") as psA:
                xt = [xtp.tile([128, S], f32r, tag=f"xt{k}", name=f"xt{k}")
                      for k in range(KC)]

                # x block 0 before the bulky weight loads
                xch0 = []
                for j in range(4):
                    xc = xs.tile([128, D], f32, tag="xc")
                    nc.sync.dma_start(xc[:], X[ts(j, 128), :])
                    xch0.append(xc)

                # ---- remaining persistent operands -----------------------
                wqt = sb.tile([128, KC * DL], f32r, tag="wqt")
                nc.sync.dma_start(wqt[:].rearrange("p (c m) -> p c m", c=KC),
                                  WQ.ap().rearrange("(c p) m -> p c m", p=128))
                wkt = sb.tile([128, KC * DL], f32r, tag="wkt")
                nc.sync.dma_start(wkt[:].rearrange("p (c m) -> p c m", c=KC),
                                  WK.ap().rearrange("(c p) m -> p c m", p=128))
                wvt = sb.tile([128, KC * DL], f32r, tag="wvt")
                nc.sync.dma_start(wvt[:].rearrange("p (c m) -> p c m", c=KC),
                                  WV.ap().rearrange("(c p) m -> p c m", p=128))
                bq2 = sb.tile([128, MC], f32, tag="bq2")
                nc.sync.dma_start(bq2[:], BQ[:, :])
                bk2 = sb.tile([128, MC], f32, tag="bk2")
                nc.sync.dma_start(bk2[:], BK[:, :])
                bv_t = sb.tile([1, DL], f32r, tag="bv")
                nc.sync.dma_start(bv_t[:], BV[:, :])
                ones_t = sb.tile([1, 128], f32r, tag="ones")
                nc.sync.dma_start(ones_t[:], ONE[:, :])
                maskT = sb.tile([128, SC], f32, tag="maskT")
                nc.sync.dma_start(maskT[:],
                                  MSK.ap().rearrange("(c p) -> p c", p=128))
                binm = sb.tile([128, SC], f32, tag="binm")
                nc.vector.tensor_scalar(binm[:], maskT[:], 0.0, None, OP.is_ge)

                # ---- phase A: transpose x, projections -------------------
                for sb_i in range(S // SQW):
                    if sb_i == 0:
                        xch = xch0
                    else:
                        xch = []
                        for j in range(4):
                            sc = sb_i * 4 + j
                            xc = xs.tile([128, D], f32, tag="xc")
                            nc.sync.dma_start(xc[:], X[ts(sc, 128), :])
                            xch.append(xc)
                    # transpose this 512-row block of x into xT
                    for kc in range(KC):
                        pt = psA.tile([128, 512], f32, tag="sc")
                        for j in range(4):
                            nc.tensor.transpose(pt[:, ts(j, 128)],
                                                xch[j][:, ts(kc, 128)], ident[:])
                        if eng == 0:
                            nc.vector.tensor_copy(xt[kc][:, ts(sb_i, 512)], pt[:])
                        else:
                            nc.scalar.copy(xt[kc][:, ts(sb_i, 512)], pt[:])
                        eng ^= 1
                    # q/k projections for this s-block (outputs transposed)
                    for mc in range(MC):
                        for w_t, bias2, dst in ((wqt, bq2, qT), (wkt, bk2, kT)):
                            pt = psA.tile([128, 512], f32, tag="sc")
                            for kc in range(KC):
                                nc.tensor.matmul(
                                    pt[:],
                                    w_t[:, kc * DL + mc * 128:
                                        kc * DL + mc * 128 + 128],
                                    xt[kc][:, ts(sb_i, 512)],
                                    start=(kc == 0), stop=(kc == KC - 1))
                            if eng == 0:
                                nc.vector.tensor_scalar_add(
                                    dst[mc][:, ts(sb_i, 512)], pt[:],
                                    bias2[:, mc:mc + 1])
                            else:
                                nc.scalar.add(dst[mc][:, ts(sb_i, 512)], pt[:],
                                              bias2[:, mc:mc + 1])
                            eng ^= 1
                    # v projection in natural layout + bias + key mask
                    for j in range(4):
                        sc = sb_i * 4 + j
                        pv = psA.tile([128, 512], f32, tag="sc")
                        nc.tensor.matmul(pv[:, 0:DL], ones_t[:], bv_t[:],
                                         start=True, stop=False)
                        for kc in range(KC):
                            nc.tensor.matmul(pv[:, 0:DL],
                                             xt[kc][:, ts(sc, 128)],
                                             wvt[:, ts(kc, DL)],
                                             start=False, stop=(kc == KC - 1))
                        if eng == 0:
                            nc.vector.tensor_scalar_mul(v_sb[:, ts(sc, DL)],
                                                        pv[:, 0:DL],
                                                        binm[:, sc:sc + 1])
                        else:
                            nc.scalar.activation(v_sb[:, ts(sc, DL)],
                                                 pv[:, 0:DL],
                                                 AF.Copy,
                                                 scale=binm[:, sc:sc + 1])
                        eng ^= 1

            # ---- phase B: attention --------------------------------------
            with tc.tile_pool(name="probs", bufs=20) as pp, \
                 tc.tile_pool(name="psB", bufs=3, space="PSUM") as psB, \
                 tc.tile_pool(name="psc", bufs=2, space="PSUM") as psc:
                for hp in range(MC):
                    for sq in range(NSQ):
                        pbs = []
                        for sk in range(SC):
                            spt = psB.tile([128, 1024], f32, tag="sc2")
                            nc.tensor.matmul(spt[:, 0:512],
                                             kT[hp][0:64, ts(sk, 128)],
                                             qT[hp][0:64, ts(sq, SQW)],
                                             start=True, stop=True)
                            nc.tensor.matmul(spt[:, 512:1024],
                                             kT[hp][64:128, ts(sk, 128)],
                                             qT[hp][64:128, ts(sq, SQW)],
                                             start=True, stop=True)
                            pb = pp.tile([128, 1024], f32r, tag="pb")
                            if eng == 0:
                                nc.vector.tensor_copy(pb[:], spt[:])
                            else:
                                nc.scalar.copy(pb[:], spt[:])
                            eng ^= 1
                            pbs.append(pb)
                        cts = [psc.tile([64, SQW], f32, tag="ctx",
                                        name=f"ct{hp}_{sq}_{i}")
                               for i in range(2)]
                        for sk in range(SC):
                            for h in range(2):
                                nc.tensor.matmul(
                                    cts[h][:],
                                    v_sb[:, sk * DL + hp * 128 + h * 64:
                                         sk * DL + hp * 128 + h * 64 + 64],
                                    pbs[sk][:, h * 512:(h + 1) * 512],
                                    start=(sk == 0), stop=(sk == SC - 1))
                        stage = stg.tile([128, SQW], f32, tag="st")
                        nc.vector.tensor_copy(stage[0:64, :], cts[0][:])
                        nc.scalar.copy(stage[64:128, :], cts[1][:])
                        nc.sync.dma_start(
                            OUT[hp * 128:(hp + 1) * 128, ts(sq, SQW)], stage[:])

    nc.compile()
    return nc


def _get_nc():
    if "nc" not in _cache:
        _cache["nc"] = _build()
    return _cache["nc"]


def _make_in_maps(hidden_states, attention_mask, Wq, bq, Wk, bk, Wv, bv):
    hs = np.ascontiguousarray(np.asarray(hidden_states, dtype=np.float32))
    am = np.asarray(attention_mask, dtype=np.float32)
    ones = np.ones((1, 128), np.float32)
    idn = np.eye(128, dtype=np.float32)
    in_maps = []
    for c in range(N_CORES):
        b, g = divmod(c, 4)
        sl = slice(g * DL, (g + 1) * DL)
        in_maps.append({
            "x": hs[b],
            "idn": idn,
            "wqt": np.ascontiguousarray(np.asarray(Wq, np.float32)[sl, :].T),
            "wkt": np.ascontiguousarray(np.asarray(Wk, np.float32)[sl, :].T),
            "wvt": np.ascontiguousarray(np.asarray(Wv, np.float32)[sl, :].T),
            "bq2": np.ascontiguousarray(
                np.asarray(bq, np.float32)[sl].reshape(MC, 128).T),
            "bk2": np.ascontiguousarray(
                np.asarray(bk, np.float32)[sl].reshape(MC, 128).T),
            "bv": np.ascontiguousarray(
                np.asarray(bv, np.float32)[sl].reshape(1, DL)),
            "ones": ones,
            "mask": np.ascontiguousarray(am[b, 0, 0, :]),
        })
    return in_maps


def _gather(results):
    out = np.empty((B, S, D), np.float32)
    for c in range(N_CORES):
        b, g = divmod(c, 4)
        out[b, :, g * DL:(g + 1) * DL] = results[c]["out"].T
    return out


def run_sharded(in_maps, **kw):
    nc = _get_nc()
    return run_bass_kernel_spmd(nc, in_maps, core_ids=list(range(N_CORES)), **kw)


def kernel(hidden_states, attention_mask, Wq, bq, Wk, bk, Wv, bv):
    in_maps = _make_in_maps(hidden_states, attention_mask,
                            Wq, bq, Wk, bk, Wv, bv)
    res = run_sharded(in_maps)
    return _gather(res.results)


# revision 4
# speedup vs baseline: 1.3731x; 1.2511x over previous
"""BertLinearSelfAttention on 8 Trainium2 NeuronCores.

Problem (per reference):
  q = hs @ Wq.T + bq ; k = hs @ Wk.T + bk ; v = hs @ Wv.T + bv   (B,S,D)
  per head: scores = q @ k.T ; probs = scores * (mask >= 0) ; ctx = probs @ v
  B=2, S=2048, D=1024, H=16, HD=64. No softmax, binary key mask.

Sharding: core c = 4*b + g handles batch b and head group g (4 heads,
256 output features). Each core runs the same SPMD program on its own
slice; output is gathered host-side.

Algebraic moves:
  1) (scores * mask_k) @ v == scores @ (mask_k * v): the binary key mask
     applies to V rows instead of the S x S probs matrix.
  2) Masked keys contribute exactly zero, so K/V work only needs the
     valid keys. Inputs are compacted to CAP key slots (valid indices +
     zero-padding); a full-width fallback program handles the
     (astronomically unlikely) case of more than CAP valid keys.

On-chip layout (per core):
  xT blocks [128, 512]  hidden transposed via PE-identity transposes
  qT [256, S], kT [256, CAP] projection outputs kept feature-major
  v  [CAP, 256]  natural layout, bias via K=1 ones-matmul, pad mask
                 applied on the PSUM->SBUF copy
  scoresT pair tiles [s_k 128, 2 x s_q 512] = kT.T @ qT for both heads
                 of a pair (K=64 packed via disjoint PE row groups, two
                 PSUM banks), drained by one wide copy
  ctxT [64, s_q] accumulated over s_k chunks (fp32r, M=64)
All matmuls run as float32r (TF32-like, ~1.5e-4 rel err, 1 cyc/row);
transposes are exact fp32.
"""
import numpy as np
import concourse.bass as bass
import concourse.mybir as mybir
import concourse.tile as tile
from concourse import bacc
from concourse.bass import ts
from concourse.bass_utils import run_bass_kernel_spmd

f32 = mybir.dt.float32
f32r = mybir.dt.float32r
AF = mybir.ActivationFunctionType

B = 2
S = 2048
D = 1024
DL = 256          # output features per core (4 heads x 64)
KC = D // 128     # 8 contraction chunks
SC = S // 128     # 16 sequence chunks
MC = DL // 128    # 2 feature chunks / head pairs
SQW = 512         # attention s_q strip width
NSQ = S // SQW    # 4 strips
N_CORES = 8
CAP = 1280        # compacted key slots (valid count ~Binom(2048,.5);
                  # 1280 is ~11 sigma above the mean, fallback covers more)

_cache = {}


def _blocks(width):
    """Split `width` columns into 512-wide blocks (last may be shorter)."""
    out = []
    off = 0
    while off < width:
        w = min(512, width - off)
        out.append((off, w))
        off += w
    return out


def _build(compact):
    skv = (CAP if compact else S) // 128   # key chunks
    nc = bacc.Bacc("TRN2", target_bir_lowering=False, debug=False,
                   num_devices=N_CORES)
    X = nc.declare_dram_parameter("x", [S, D], f32, isOutput=False)
    if compact:
        XKV = nc.declare_dram_parameter("xkv", [CAP, D], f32, isOutput=False)
    IDN = nc.declare_dram_parameter("idn", [128, 128], f32, isOutput=False)
    WQ = nc.declare_dram_parameter("wqt", [D, DL], f32r, isOutput=False)
    WK = nc.declare_dram_parameter("wkt", [D, DL], f32r, isOutput=False)
    WV = nc.declare_dram_parameter("wvt", [D, DL], f32r, isOutput=False)
    BQ = nc.declare_dram_parameter("bq2", [128, MC], f32, isOutput=False)
    BK = nc.declare_dram_parameter("bk2", [128, MC], f32, isOutput=False)
    BV = nc.declare_dram_parameter("bv", [1, DL], f32r, isOutput=False)
    ONE = nc.declare_dram_parameter("ones", [1, 128], f32r, isOutput=False)
    KVM = nc.declare_dram_parameter("kvm2", [128, skv], f32, isOutput=False)
    OUT = nc.declare_dram_parameter("out", [DL, S], f32, isOutput=True)

    with tile.TileContext(nc) as tc:
        with tc.tile_pool(name="sb", bufs=1) as sb, \
             tc.tile_pool(name="stg", bufs=4) as stg:

            ident = sb.tile([128, 128], f32, tag="ident")
            nc.sync.dma_start(ident[:], IDN[:, :])

            qT = [sb.tile([128, S], f32r, tag=f"qT{m}", name=f"qT{m}")
                  for m in range(MC)]
            kT = [sb.tile([128, CAP if compact else S], f32r,
                          tag=f"kT{m}", name=f"kT{m}") for m in range(MC)]
            v_sb = sb.tile([128, skv * DL], f32r, tag="v_sb")

            eng = 0  # DVE/ACT alternator for PSUM->SBUF drains

            with tc.tile_pool(name="xs", bufs=6) as xs, \
                 tc.tile_pool(name="xtp", bufs=2) as xtp, \
                 tc.tile_pool(name="psA", bufs=6, space="PSUM") as psA:

                # first x block queued before the bulky weight loads
                xch0 = []
                for j in range(4):
                    xc = xs.tile([128, D], f32, tag="xc")
                    nc.sync.dma_start(xc[:], X[ts(j, 128), :])
                    xch0.append(xc)

                wqt = sb.tile([128, KC * DL], f32r, tag="wqt")
                nc.sync.dma_start(wqt[:].rearrange("p (c m) -> p c m", c=KC),
                                  WQ.ap().rearrange("(c p) m -> p c m", p=128))
                wkt = sb.tile([128, KC * DL], f32r, tag="wkt")
                nc.sync.dma_start(wkt[:].rearrange("p (c m) -> p c m", c=KC),
                                  WK.ap().rearrange("(c p) m -> p c m", p=128))
                wvt = sb.tile([128, KC * DL], f32r, tag="wvt")
                nc.sync.dma_start(wvt[:].rearrange("p (c m) -> p c m", c=KC),
                                  WV.ap().rearrange("(c p) m -> p c m", p=128))
                bq2 = sb.tile([128, MC], f32, tag="bq2")
                nc.sync.dma_start(bq2[:], BQ[:, :])
                bk2 = sb.tile([128, MC], f32, tag="bk2")
                nc.sync.dma_start(bk2[:], BK[:, :])
                bv_t = sb.tile([1, DL], f32r, tag="bv")
                nc.sync.dma_start(bv_t[:], BV[:, :])
                ones_t = sb.tile([1, 128], f32r, tag="ones")
                nc.sync.dma_start(ones_t[:], ONE[:, :])
                kvm = sb.tile([128, skv], f32, tag="kvm")
                nc.sync.dma_start(kvm[:], KVM[:, :])

                def transpose_block(src_tiles, col0, width, kc, dst, dst_off):
                    """PE-transpose width cols of chunk tiles into dst."""
                    pt = psA.tile([128, 512], f32, tag="sc", name="tp")
                    nw = width // 128
                    for j in range(nw):
                        nc.tensor.transpose(pt[:, ts(j, 128)],
                                            src_tiles[j][:, ts(kc, 128)],
                                            ident[:])
                    return pt

                def drain(dst_ap, src_ap, bias=None, scale=None):
                    nonlocal eng
                    if eng == 0:
                        if bias is not None:
                            nc.vector.tensor_scalar_add(dst_ap, src_ap, bias)
                        elif scale is not None:
                            nc.vector.tensor_scalar_mul(dst_ap, src_ap, scale)
                        else:
                            nc.vector.tensor_copy(dst_ap, src_ap)
                    else:
                        if bias is not None:
                            nc.scalar.add(dst_ap, src_ap, bias)
                        elif scale is not None:
                            nc.scalar.activation(dst_ap, src_ap, AF.Copy,
                                                 scale=scale)
                        else:
                            nc.scalar.copy(dst_ap, src_ap)
                    eng ^= 1

                # ---- A1: Q path over full x ------------------------------
                for bi, (off, w) in enumerate(_blocks(S)):
                    if bi == 0:
                        xch = xch0
                    else:
                        xch = []
                        for j in range(w // 128):
                            xc = xs.tile([128, D], f32, tag="xc")
                            nc.sync.dma_start(xc[:],
                                              X[off + j * 128:off + (j + 1) * 128, :])
                            xch.append(xc)
                    xtb = []
                    for kc in range(KC):
                        pt = transpose_block(xch, off, w, kc, None, None)
                        xb = xtp.tile([128, 512], f32r, tag=f"xt{kc}",
                                      name=f"xt{kc}")
                        drain(xb[:, 0:w], pt[:, 0:w])
                        xtb.append(xb)
                    for mc in range(MC):
                        pt = psA.tile([128, 512], f32, tag="sc", name="qp")
                        for kc in range(KC):
                            nc.tensor.matmul(
                                pt[:, 0:w],
                                wqt[:, kc * DL + mc * 128:kc * DL + mc * 128 + 128],
                                xtb[kc][:, 0:w],
                                start=(kc == 0), stop=(kc == KC - 1))
                        drain(qT[mc][:, off:off + w], pt[:, 0:w],
                              bias=bq2[:, mc:mc + 1])
                    if not compact:
                        # K/V share the same transposed blocks
                        for mc in range(MC):
                            pt = psA.tile([128, 512], f32, tag="sc", name="kp")
                            for kc in range(KC):
                                nc.tensor.matmul(
                                    pt[:, 0:w],
                                    wkt[:, kc * DL + mc * 128:kc * DL + mc * 128 + 128],
                                    xtb[kc][:, 0:w],
                                    start=(kc == 0), stop=(kc == KC - 1))
                            drain(kT[mc][:, off:off + w], pt[:, 0:w],
                                  bias=bk2[:, mc:mc + 1])
                        for j in range(w // 128):
                            sc = (off + j * 128) // 128
                            pv = psA.tile([128, 512], f32, tag="sc", name="vp")
                            nc.tensor.matmul(pv[:, 0:DL], ones_t[:], bv_t[:],
                                             start=True, stop=False)
                            for kc in range(KC):
                                nc.tensor.matmul(pv[:, 0:DL],
                                                 xtb[kc][:, ts(j, 128)],
                                                 wvt[:, ts(kc, DL)],
                                                 start=False,
                                                 stop=(kc == KC - 1))
                            drain(v_sb[:, ts(sc, DL)], pv[:, 0:DL],
                                  scale=kvm[:, sc:sc + 1])

                # ---- A2 (compact): K/V over gathered keys ----------------
                if compact:
                    for off, w in _blocks(CAP):
                        xch = []
                        for j in range(w // 128):
                            xc = xs.tile([128, D], f32, tag="xc")
                            nc.sync.dma_start(
                                xc[:],
                                XKV[off + j * 128:off + (j + 1) * 128, :])
                            xch.append(xc)
                        xtb = []
                        for kc in range(KC):
                            pt = transpose_block(xch, off, w, kc, None, None)
                            xb = xtp.tile([128, 512], f32r, tag=f"xt{kc}",
                                          name=f"xkvt{kc}")
                            drain(xb[:, 0:w], pt[:, 0:w])
                            xtb.append(xb)
                        for mc in range(MC):
                            pt = psA.tile([128, 512], f32, tag="sc", name="kp")
                            for kc in range(KC):
                                nc.tensor.matmul(
                                    pt[:, 0:w],
                                    wkt[:, kc * DL + mc * 128:kc * DL + mc * 128 + 128],
                                    xtb[kc][:, 0:w],
                                    start=(kc == 0), stop=(kc == KC - 1))
                            drain(kT[mc][:, off:off + w], pt[:, 0:w],
                                  bias=bk2[:, mc:mc + 1])
                        for j in range(w // 128):
                            sc = (off + j * 128) // 128
                            pv = psA.tile([128, 512], f32, tag="sc", name="vp")
                            nc.tensor.matmul(pv[:, 0:DL], ones_t[:], bv_t[:],
                                             start=True, stop=False)
                            for kc in range(KC):
                                nc.tensor.matmul(pv[:, 0:DL],
                                                 xtb[kc][:, ts(j, 128)],
                                                 wvt[:, ts(kc, DL)],
                                                 start=False,
                                                 stop=(kc == KC - 1))
                            drain(v_sb[:, ts(sc, DL)], pv[:, 0:DL],
                                  scale=kvm[:, sc:sc + 1])

            # ---- phase B: attention --------------------------------------
            with tc.tile_pool(name="probs", bufs=skv + 4) as pp, \
                 tc.tile_pool(name="psB", bufs=3, space="PSUM") as psB, \
                 tc.tile_pool(name="psc", bufs=2, space="PSUM") as psc:
                for hp in range(MC):
                    for sq in range(NSQ):
                        pbs = []
                        for sk in range(skv):
                            spt = psB.tile([128, 1024], f32, tag="sc2")
                            nc.tensor.matmul(spt[:, 0:512],
                                             kT[hp][0:64, ts(sk, 128)],
                                             qT[hp][0:64, ts(sq, SQW)],
                                             start=True, stop=True)
                            nc.tensor.matmul(spt[:, 512:1024],
                                             kT[hp][64:128, ts(sk, 128)],
                                             qT[hp][64:128, ts(sq, SQW)],
                                             start=True, stop=True)
                            pb = pp.tile([128, 1024], f32r, tag="pb")
                            if eng == 0:
                                nc.vector.tensor_copy(pb[:], spt[:])
                            else:
                                nc.scalar.copy(pb[:], spt[:])
                            eng ^= 1
                            pbs.append(pb)
                        cts = [psc.tile([64, SQW], f32, tag="ctx",
                                        name=f"ct{hp}_{sq}_{i}")
                               for i in range(2)]
                        for sk in range(skv):
                            for h in range(2):
                                nc.tensor.matmul(
                                    cts[h][:],
                                    v_sb[:, sk * DL + hp * 128 + h * 64:
                                         sk * DL + hp * 128 + h * 64 + 64],
                                    pbs[sk][:, h * 512:(h + 1) * 512],
                                    start=(sk == 0), stop=(sk == skv - 1))
                        stage = stg.tile([128, SQW], f32, tag="st")
                        nc.vector.tensor_copy(stage[0:64, :], cts[0][:])
                        nc.scalar.copy(stage[64:128, :], cts[1][:])
                        nc.sync.dma_start(
                            OUT[hp * 128:(hp + 1) * 128, ts(sq, SQW)], stage[:])

    nc.compile()
    return nc


def _get_nc(compact):
    key = "compact" if compact else "full"
    if key not in _cache:
        _cache[key] = _build(compact)
    return _cache[key]


def _make_in_maps(hidden_states, attention_mask, Wq, bq, Wk, bk, Wv, bv):
    hs = np.ascontiguousarray(np.asarray(hidden_states, dtype=np.float32))
    am = np.asarray(attention_mask, dtype=np.float32)

    # key compaction metadata per batch
    compact = True
    idxs, kvms, xkvs = [], [], []
    for b in range(B):
        valid = np.nonzero(am[b, 0, 0, :] >= 0)[0]
        if len(valid) > CAP:
            compact = False
            break
        idxp = np.zeros(CAP, np.int64)
        idxp[:len(valid)] = valid
        kvm = np.zeros(CAP, np.float32)
        kvm[:len(valid)] = 1.0
        idxs.append(idxp)
        kvms.append(kvm)
        xkvs.append(np.ascontiguousarray(hs[b][idxp]))

    skv = (CAP if compact else S) // 128
    ones = np.ones((1, 128), np.float32)
    idn = np.eye(128, dtype=np.float32)
    in_maps = []
    for c in range(N_CORES):
        b, g = divmod(c, 4)
        sl = slice(g * DL, (g + 1) * DL)
        if compact:
            kvm2 = np.ascontiguousarray(kvms[b].reshape(skv, 128).T)
        else:
            kvm2 = np.ascontiguousarray(
                (am[b, 0, 0, :] >= 0).astype(np.float32).reshape(skv, 128).T)
        m = {
            "x": hs[b],
            "idn": idn,
            "wqt": np.ascontiguousarray(np.asarray(Wq, np.float32)[sl, :].T),
            "wkt": np.ascontiguousarray(np.asarray(Wk, np.float32)[sl, :].T),
            "wvt": np.ascontiguousarray(np.asarray(Wv, np.float32)[sl, :].T),
            "bq2": np.ascontiguousarray(
                np.asarray(bq, np.float32)[sl].reshape(MC, 128).T),
            "bk2": np.ascontiguousarray(
                np.asarray(bk, np.float32)[sl].reshape(MC, 128).T),
            "bv": np.ascontiguousarray(
                np.asarray(bv, np.float32)[sl].reshape(1, DL)),
            "ones": ones,
            "kvm2": kvm2,
        }
        if compact:
            m["xkv"] = xkvs[b]
        in_maps.append(m)
    return compact, in_maps


def _gather(results):
    out = np.empty((B, S, D), np.float32)
    for c in range(N_CORES):
        b, g = divmod(c, 4)
        out[b, :, g * DL:(g + 1) * DL] = results[c]["out"].T
    return out


def run_sharded(compact, in_maps, **kw):
    nc = _get_nc(compact)
    return run_bass_kernel_spmd(nc, in_maps, core_ids=list(range(N_CORES)), **kw)


def kernel(hidden_states, attention_mask, Wq, bq, Wk, bk, Wv, bv):
    compact, in_maps = _make_in_maps(hidden_states, attention_mask,
                                     Wq, bq, Wk, bk, Wv, bv)
    res = run_sharded(compact, in_maps)
    return _gather(res.results)


# revision 5
# speedup vs baseline: 1.5209x; 1.1076x over previous
"""BertLinearSelfAttention on 8 Trainium2 NeuronCores.

Problem (per reference):
  q = hs @ Wq.T + bq ; k = hs @ Wk.T + bk ; v = hs @ Wv.T + bv   (B,S,D)
  per head: scores = q @ k.T ; probs = scores * (mask >= 0) ; ctx = probs @ v
  B=2, S=2048, D=1024, H=16, HD=64. No softmax, binary key mask.

Sharding: core c = 4*b + g handles batch b and head group g (4 heads,
256 output features). Each core runs the same SPMD program on its own
slice; output is gathered host-side.

Algebraic moves:
  1) (scores * mask_k) @ v == scores @ (mask_k * v): the binary key mask
     applies to V rows instead of the S x S probs matrix.
  2) Masked keys contribute exactly zero, so K/V work only needs the
     valid keys. Inputs are compacted to CAP key slots (valid indices +
     zero-padding); a full-width fallback program handles the
     (astronomically unlikely) case of more than CAP valid keys.

On-chip layout (per core):
  xT blocks [128, 512]  hidden transposed via PE-identity transposes
  qT [256, S], kT [256, CAP] projection outputs kept feature-major
  v  [CAP, 256]  natural layout, bias via K=1 ones-matmul, pad mask
                 applied on the PSUM->SBUF copy
  scoresT pair tiles [s_k 128, 2 x s_q 512] = kT.T @ qT for both heads
                 of a pair (K=64 packed via disjoint PE row groups, two
                 PSUM banks), drained by one wide copy
  ctxT [64, s_q] accumulated over s_k chunks (fp32r, M=64)
All matmuls run as float32r (TF32-like, ~1.5e-4 rel err, 1 cyc/row);
transposes are exact fp32.
"""
import numpy as np
import concourse.bass as bass
import concourse.mybir as mybir
import concourse.tile as tile
from concourse import bacc
from concourse.bass import ts
from concourse.bass_utils import run_bass_kernel_spmd

f32 = mybir.dt.float32
f32r = mybir.dt.float32r
bf16 = mybir.dt.bfloat16
AF = mybir.ActivationFunctionType

B = 2
S = 2048
D = 1024
DL = 256          # output features per core (4 heads x 64)
KC = D // 128     # 8 contraction chunks
SC = S // 128     # 16 sequence chunks
MC = DL // 128    # 2 feature chunks / head pairs
SQW = 512         # attention s_q strip width
NSQ = S // SQW    # 4 strips
N_CORES = 8
CAP = 1152        # compacted key slots (valid count ~Binom(2048,.5), mean
                  # 1024 sd 22.6; 1152 is ~5.7 sigma up, fallback covers more)
CTX_BF16 = True   # probs/v in bf16 -> ctx pairs col-packed in the PE array

_cache = {}


def _blocks(width):
    """Split `width` columns into 512-wide blocks (last may be shorter)."""
    out = []
    off = 0
    while off < width:
        w = min(512, width - off)
        out.append((off, w))
        off += w
    return out


def _build(compact):
    skv = (CAP if compact else S) // 128   # key chunks
    nc = bacc.Bacc("TRN2", target_bir_lowering=False, debug=False,
                   num_devices=N_CORES)
    X = nc.declare_dram_parameter("x", [S, D], f32, isOutput=False)
    if compact:
        XKV = nc.declare_dram_parameter("xkv", [CAP, D], f32, isOutput=False)
    IDN = nc.declare_dram_parameter("idn", [128, 128], f32, isOutput=False)
    WQ = nc.declare_dram_parameter("wqt", [D, DL], f32r, isOutput=False)
    WK = nc.declare_dram_parameter("wkt", [D, DL], f32r, isOutput=False)
    WV = nc.declare_dram_parameter("wvt", [D, DL], f32r, isOutput=False)
    BQ = nc.declare_dram_parameter("bq2", [128, MC], f32, isOutput=False)
    BK = nc.declare_dram_parameter("bk2", [128, MC], f32, isOutput=False)
    BV = nc.declare_dram_parameter("bv", [1, DL], f32r, isOutput=False)
    ONE = nc.declare_dram_parameter("ones", [1, 128], f32r, isOutput=False)
    KVM = nc.declare_dram_parameter("kvm2", [128, skv], f32, isOutput=False)
    OUT = nc.declare_dram_parameter("out", [DL, S], f32, isOutput=True)

    with tile.TileContext(nc) as tc:
        with tc.tile_pool(name="sb", bufs=1) as sb, \
             tc.tile_pool(name="stg", bufs=4) as stg:

            ident = sb.tile([128, 128], f32, tag="ident")
            nc.sync.dma_start(ident[:], IDN[:, :])

            qT = [sb.tile([128, S], f32r, tag=f"qT{m}", name=f"qT{m}")
                  for m in range(MC)]
            kT = [sb.tile([128, CAP if compact else S], f32r,
                          tag=f"kT{m}", name=f"kT{m}") for m in range(MC)]
            v_sb = sb.tile([128, skv * DL], bf16 if CTX_BF16 else f32r, tag="v_sb")

            eng = 0  # DVE/ACT alternator for PSUM->SBUF drains

            with tc.tile_pool(name="xs", bufs=6) as xs, \
                 tc.tile_pool(name="xtp", bufs=2) as xtp, \
                 tc.tile_pool(name="psA", bufs=6, space="PSUM") as psA:

                # first x block queued before the bulky weight loads
                xch0 = []
                for j in range(4):
                    xc = xs.tile([128, D], f32, tag="xc")
                    nc.sync.dma_start(xc[:], X[ts(j, 128), :])
                    xch0.append(xc)

                wqt = sb.tile([128, KC * DL], f32r, tag="wqt")
                nc.sync.dma_start(wqt[:].rearrange("p (c m) -> p c m", c=KC),
                                  WQ.ap().rearrange("(c p) m -> p c m", p=128))
                wkt = sb.tile([128, KC * DL], f32r, tag="wkt")
                nc.sync.dma_start(wkt[:].rearrange("p (c m) -> p c m", c=KC),
                                  WK.ap().rearrange("(c p) m -> p c m", p=128))
                wvt = sb.tile([128, KC * DL], f32r, tag="wvt")
                nc.sync.dma_start(wvt[:].rearrange("p (c m) -> p c m", c=KC),
                                  WV.ap().rearrange("(c p) m -> p c m", p=128))
                bq2 = sb.tile([128, MC], f32, tag="bq2")
                nc.sync.dma_start(bq2[:], BQ[:, :])
                bk2 = sb.tile([128, MC], f32, tag="bk2")
                nc.sync.dma_start(bk2[:], BK[:, :])
                bv_t = sb.tile([1, DL], f32r, tag="bv")
                nc.sync.dma_start(bv_t[:], BV[:, :])
                ones_t = sb.tile([1, 128], f32r, tag="ones")
                nc.sync.dma_start(ones_t[:], ONE[:, :])
                kvm = sb.tile([128, skv], f32, tag="kvm")
                nc.sync.dma_start(kvm[:], KVM[:, :])

                def transpose_block(src_tiles, col0, width, kc, dst, dst_off):
                    """PE-transpose width cols of chunk tiles into dst."""
                    pt = psA.tile([128, 512], f32, tag="sc", name="tp")
                    nw = width // 128
                    for j in range(nw):
                        nc.tensor.transpose(pt[:, ts(j, 128)],
                                            src_tiles[j][:, ts(kc, 128)],
                                            ident[:])
                    return pt

                def drain(dst_ap, src_ap, bias=None, scale=None):
                    nonlocal eng
                    if eng == 0:
                        if bias is not None:
                            nc.vector.tensor_scalar_add(dst_ap, src_ap, bias)
                        elif scale is not None:
                            nc.vector.tensor_scalar_mul(dst_ap, src_ap, scale)
                        else:
                            nc.vector.tensor_copy(dst_ap, src_ap)
                    else:
                        if bias is not None:
                            nc.scalar.add(dst_ap, src_ap, bias)
                        elif scale is not None:
                            nc.scalar.activation(dst_ap, src_ap, AF.Copy,
                                                 scale=scale)
                        else:
                            nc.scalar.copy(dst_ap, src_ap)
                    eng ^= 1

                # ---- A1: Q path over full x ------------------------------
                for bi, (off, w) in enumerate(_blocks(S)):
                    if bi == 0:
                        xch = xch0
                    else:
                        xch = []
                        for j in range(w // 128):
                            xc = xs.tile([128, D], f32, tag="xc")
                            nc.sync.dma_start(xc[:],
                                              X[off + j * 128:off + (j + 1) * 128, :])
                            xch.append(xc)
                    xtb = []
                    for kc in range(KC):
                        pt = transpose_block(xch, off, w, kc, None, None)
                        xb = xtp.tile([128, 512], f32r, tag=f"xt{kc}",
                                      name=f"xt{kc}")
                        drain(xb[:, 0:w], pt[:, 0:w])
                        xtb.append(xb)
                    for mc in range(MC):
                        pt = psA.tile([128, 512], f32, tag="sc", name="qp")
                        for kc in range(KC):
                            nc.tensor.matmul(
                                pt[:, 0:w],
                                wqt[:, kc * DL + mc * 128:kc * DL + mc * 128 + 128],
                                xtb[kc][:, 0:w],
                                start=(kc == 0), stop=(kc == KC - 1))
                        drain(qT[mc][:, off:off + w], pt[:, 0:w],
                              bias=bq2[:, mc:mc + 1])
                    if not compact:
                        # K/V share the same transposed blocks
                        for mc in range(MC):
                            pt = psA.tile([128, 512], f32, tag="sc", name="kp")
                            for kc in range(KC):
                                nc.tensor.matmul(
                                    pt[:, 0:w],
                                    wkt[:, kc * DL + mc * 128:kc * DL + mc * 128 + 128],
                                    xtb[kc][:, 0:w],
                                    start=(kc == 0), stop=(kc == KC - 1))
                            drain(kT[mc][:, off:off + w], pt[:, 0:w],
                                  bias=bk2[:, mc:mc + 1])
                        for j in range(w // 128):
                            sc = (off + j * 128) // 128
                            pv = psA.tile([128, 512], f32, tag="sc", name="vp")
                            nc.tensor.matmul(pv[:, 0:DL], ones_t[:], bv_t[:],
                                             start=True, stop=False)
                            for kc in range(KC):
                                nc.tensor.matmul(pv[:, 0:DL],
                                                 xtb[kc][:, ts(j, 128)],
                                                 wvt[:, ts(kc, DL)],
                                                 start=False,
                                                 stop=(kc == KC - 1))
                            drain(v_sb[:, ts(sc, DL)], pv[:, 0:DL],
                                  scale=kvm[:, sc:sc + 1])

                # ---- A2 (compact): K/V over gathered keys ----------------
                if compact:
                    for off, w in _blocks(CAP):
                        xch = []
                        for j in range(w // 128):
                            xc = xs.tile([128, D], f32, tag="xc")
                            nc.sync.dma_start(
                                xc[:],
                                XKV[off + j * 128:off + (j + 1) * 128, :])
                            xch.append(xc)
                        xtb = []
                        for kc in range(KC):
                            pt = transpose_block(xch, off, w, kc, None, None)
                            xb = xtp.tile([128, 512], f32r, tag=f"xt{kc}",
                                          name=f"xkvt{kc}")
                            drain(xb[:, 0:w], pt[:, 0:w])
                            xtb.append(xb)
                        for mc in range(MC):
                            pt = psA.tile([128, 512], f32, tag="sc", name="kp")
                            for kc in range(KC):
                                nc.tensor.matmul(
                                    pt[:, 0:w],
                                    wkt[:, kc * DL + mc * 128:kc * DL + mc * 128 + 128],
                                    xtb[kc][:, 0:w],
                                    start=(kc == 0), stop=(kc == KC - 1))
                            drain(kT[mc][:, off:off + w], pt[:, 0:w],
                                  bias=bk2[:, mc:mc + 1])
                        for j in range(w // 128):
                            sc = (off + j * 128) // 128
                            pv = psA.tile([128, 512], f32, tag="sc", name="vp")
                            nc.tensor.matmul(pv[:, 0:DL], ones_t[:], bv_t[:],
                                             start=True, stop=False)
                            for kc in range(KC):
                                nc.tensor.matmul(pv[:, 0:DL],
                                                 xtb[kc][:, ts(j, 128)],
                                                 wvt[:, ts(kc, DL)],
                                                 start=False,
                                                 stop=(kc == KC - 1))
                            drain(v_sb[:, ts(sc, DL)], pv[:, 0:DL],
                                  scale=kvm[:, sc:sc + 1])

            # ---- phase B: attention --------------------------------------
            with tc.tile_pool(name="probs", bufs=skv + 4) as pp, \
                 tc.tile_pool(name="psB", bufs=3, space="PSUM") as psB, \
                 tc.tile_pool(name="psc", bufs=2, space="PSUM") as psc:
                for hp in range(MC):
                    for sq in range(NSQ):
                        pbs = []
                        for sk in range(skv):
                            spt = psB.tile([128, 1024], f32, tag="sc2")
                            nc.tensor.matmul(spt[:, 0:512],
                                             kT[hp][0:64, ts(sk, 128)],
                                             qT[hp][0:64, ts(sq, SQW)],
                                             start=True, stop=True)
                            nc.tensor.matmul(spt[:, 512:1024],
                                             kT[hp][64:128, ts(sk, 128)],
                                             qT[hp][64:128, ts(sq, SQW)],
                                             start=True, stop=True)
                            pb = pp.tile([128, 1024], bf16 if CTX_BF16 else f32r, tag="pb")
                            if eng == 0:
                                nc.vector.tensor_copy(pb[:], spt[:])
                            else:
                                nc.scalar.copy(pb[:], spt[:])
                            eng ^= 1
                            pbs.append(pb)
                        if CTX_BF16:
                            ct = psc.tile([128, SQW], f32, tag="ctx",
                                          name=f"ct{hp}_{sq}")
                            for sk in range(skv):
                                for h in range(2):
                                    nc.tensor.matmul(
                                        ct[h * 64:(h + 1) * 64, :],
                                        v_sb[:, sk * DL + hp * 128 + h * 64:
                                             sk * DL + hp * 128 + h * 64 + 64],
                                        pbs[sk][:, h * 512:(h + 1) * 512],
                                        start=(sk == 0), stop=(sk == skv - 1),
                                        tile_position=(0, h * 64),
                                        skip_group_check=True)
                            stage = stg.tile([128, SQW], f32, tag="st")
                            if eng == 0:
                                nc.vector.tensor_copy(stage[:], ct[:])
                            else:
                                nc.scalar.copy(stage[:], ct[:])
                            eng ^= 1
                        else:
                            cts = [psc.tile([64, SQW], f32, tag="ctx",
                                            name=f"ct{hp}_{sq}_{i}")
                                   for i in range(2)]
                            for sk in range(skv):
                                for h in range(2):
                                    nc.tensor.matmul(
                                        cts[h][:],
                                        v_sb[:, sk * DL + hp * 128 + h * 64:
                                             sk * DL + hp * 128 + h * 64 + 64],
                                        pbs[sk][:, h * 512:(h + 1) * 512],
                                        start=(sk == 0), stop=(sk == skv - 1))
                            stage = stg.tile([128, SQW], f32, tag="st")
                            nc.vector.tensor_copy(stage[0:64, :], cts[0][:])
                            nc.scalar.copy(stage[64:128, :], cts[1][:])
                        nc.sync.dma_start(
                            OUT[hp * 128:(hp + 1) * 128, ts(sq, SQW)], stage[:])

    nc.compile()
    return nc


def _get_nc(compact):
    key = "compact" if compact else "full"
    if key not in _cache:
        _cache[key] = _build(compact)
    return _cache[key]


def _make_in_maps(hidden_states, attention_mask, Wq, bq, Wk, bk, Wv, bv):
    hs = np.ascontiguousarray(np.asarray(hidden_states, dtype=np.float32))
    am = np.asarray(attention_mask, dtype=np.float32)

    # key compaction metadata per batch
    compact = True
    idxs, kvms, xkvs = [], [], []
    for b in range(B):
        valid = np.nonzero(am[b, 0, 0, :] >= 0)[0]
        if len(valid) > CAP:
            compact = False
            break
        idxp = np.zeros(CAP, np.int64)
        idxp[:len(valid)] = valid
        kvm = np.zeros(CAP, np.float32)
        kvm[:len(valid)] = 1.0
        idxs.append(idxp)
        kvms.append(kvm)
        xkvs.append(np.ascontiguousarray(hs[b][idxp]))

    skv = (CAP if compact else S) // 128
    ones = np.ones((1, 128), np.float32)
    idn = np.eye(128, dtype=np.float32)
    in_maps = []
    for c in range(N_CORES):
        b, g = divmod(c, 4)
        sl = slice(g * DL, (g + 1) * DL)
        if compact:
            kvm2 = np.ascontiguousarray(kvms[b].reshape(skv, 128).T)
        else:
            kvm2 = np.ascontiguousarray(
                (am[b, 0, 0, :] >= 0).astype(np.float32).reshape(skv, 128).T)
        m = {
            "x": hs[b],
            "idn": idn,
            "wqt": np.ascontiguousarray(np.asarray(Wq, np.float32)[sl, :].T),
            "wkt": np.ascontiguousarray(np.asarray(Wk, np.float32)[sl, :].T),
            "wvt": np.ascontiguousarray(np.asarray(Wv, np.float32)[sl, :].T),
            "bq2": np.ascontiguousarray(
                np.asarray(bq, np.float32)[sl].reshape(MC, 128).T),
            "bk2": np.ascontiguousarray(
                np.asarray(bk, np.float32)[sl].reshape(MC, 128).T),
            "bv": np.ascontiguousarray(
                np.asarray(bv, np.float32)[sl].reshape(1, DL)),
            "ones": ones,
            "kvm2": kvm2,
        }
        if compact:
            m["xkv"] = xkvs[b]
        in_maps.append(m)
    return compact, in_maps


def _gather(results):
    out = np.empty((B, S, D), np.float32)
    for c in range(N_CORES):
        b, g = divmod(c, 4)
        out[b, :, g * DL:(g + 1) * DL] = results[c]["out"].T
    return out


def run_sharded(compact, in_maps, **kw):
    nc = _get_nc(compact)
    return run_bass_kernel_spmd(nc, in_maps, core_ids=list(range(N_CORES)), **kw)


def kernel(hidden_states, attention_mask, Wq, bq, Wk, bk, Wv, bv):
    compact, in_maps = _make_in_maps(hidden_states, attention_mask,
                                     Wq, bq, Wk, bk, Wv, bv)
    res = run_sharded(compact, in_maps)
    return _gather(res.results)


# revision 6
# speedup vs baseline: 1.5675x; 1.0306x over previous
"""BertLinearSelfAttention on 8 Trainium2 NeuronCores.

Problem (per reference):
  q = hs @ Wq.T + bq ; k = hs @ Wk.T + bk ; v = hs @ Wv.T + bv   (B,S,D)
  per head: scores = q @ k.T ; probs = scores * (mask >= 0) ; ctx = probs @ v
  B=2, S=2048, D=1024, H=16, HD=64. No softmax, binary key mask.

Sharding: core c = 4*b + g handles batch b and head group g (4 heads,
256 output features). Each core runs the same SPMD program on its own
slice; output is gathered host-side.

Algebraic moves:
  1) (scores * mask_k) @ v == scores @ (mask_k * v): the binary key mask
     applies to V rows instead of the S x S probs matrix.
  2) Masked keys contribute exactly zero, so K/V work only needs the
     valid keys. Inputs are compacted to CAP key slots (valid indices +
     zero-padding); a full-width fallback program handles the
     (astronomically unlikely) case of more than CAP valid keys.

On-chip layout (per core):
  xT blocks [128, 512]  hidden transposed via PE-identity transposes
  qT [256, S], kT [256, CAP] projection outputs kept feature-major
  v  [CAP, 256]  natural layout, bias via K=1 ones-matmul, pad mask
                 applied on the PSUM->SBUF copy
  scoresT pair tiles [s_k 128, 2 x s_q 512] = kT.T @ qT for both heads
                 of a pair (K=64 packed via disjoint PE row groups, two
                 PSUM banks), drained by one wide copy
  ctxT [64, s_q] accumulated over s_k chunks (fp32r, M=64)
All matmuls run as float32r (TF32-like, ~1.5e-4 rel err, 1 cyc/row);
transposes are exact fp32.
"""
import numpy as np
import concourse.bass as bass
import concourse.mybir as mybir
import concourse.tile as tile
from concourse import bacc
from concourse.bass import ts
from concourse.bass_utils import run_bass_kernel_spmd

f32 = mybir.dt.float32
f32r = mybir.dt.float32r
bf16 = mybir.dt.bfloat16
AF = mybir.ActivationFunctionType

B = 2
S = 2048
D = 1024
DL = 256          # output features per core (4 heads x 64)
KC = D // 128     # 8 contraction chunks
SC = S // 128     # 16 sequence chunks
MC = DL // 128    # 2 feature chunks / head pairs
SQW = 512         # attention s_q strip width
NSQ = S // SQW    # 4 strips
N_CORES = 8
CAP = 1152        # compacted key slots (valid count ~Binom(2048,.5), mean
                  # 1024 sd 22.6; 1152 is ~5.7 sigma up, fallback covers more)
CTX_BF16 = True   # probs/v in fp16 -> ctx pairs col-packed in the PE array
CTX_DT = mybir.dt.float16

_cache = {}


def _blocks(width):
    """Split `width` columns into 512-wide blocks (last may be shorter)."""
    out = []
    off = 0
    while off < width:
        w = min(512, width - off)
        out.append((off, w))
        off += w
    return out


def _build(compact):
    skv = (CAP if compact else S) // 128   # key chunks
    nc = bacc.Bacc("TRN2", target_bir_lowering=False, debug=False,
                   num_devices=N_CORES)
    X = nc.declare_dram_parameter("x", [S, D], f32, isOutput=False)
    if compact:
        XKV = nc.declare_dram_parameter("xkv", [CAP, D], f32, isOutput=False)
    IDN = nc.declare_dram_parameter("idn", [128, 128], f32, isOutput=False)
    WQ = nc.declare_dram_parameter("wqt", [D, DL], f32r, isOutput=False)
    WK = nc.declare_dram_parameter("wkt", [D, DL], f32r, isOutput=False)
    WV = nc.declare_dram_parameter("wvt", [D, DL], f32r, isOutput=False)
    BQ = nc.declare_dram_parameter("bq2", [128, MC], f32, isOutput=False)
    BK = nc.declare_dram_parameter("bk2", [128, MC], f32, isOutput=False)
    BV = nc.declare_dram_parameter("bv", [1, DL], f32r, isOutput=False)
    ONE = nc.declare_dram_parameter("ones", [1, 128], f32r, isOutput=False)
    KVM = nc.declare_dram_parameter("kvm2", [128, skv], f32, isOutput=False)
    OUT = nc.declare_dram_parameter("out", [DL, S], f32, isOutput=True)

    with tile.TileContext(nc) as tc:
        with tc.tile_pool(name="sb", bufs=1) as sb, \
             tc.tile_pool(name="stg", bufs=4) as stg:

            ident = sb.tile([128, 128], f32, tag="ident")
            nc.sync.dma_start(ident[:], IDN[:, :])

            qT = [sb.tile([128, S], f32r, tag=f"qT{m}", name=f"qT{m}")
                  for m in range(MC)]
            kT = [sb.tile([128, CAP if compact else S], f32r,
                          tag=f"kT{m}", name=f"kT{m}") for m in range(MC)]
            v_sb = sb.tile([128, skv * DL], CTX_DT if CTX_BF16 else f32r, tag="v_sb")

            eng = 0  # DVE/ACT alternator for PSUM->SBUF drains

            with tc.tile_pool(name="xs", bufs=6) as xs, \
                 tc.tile_pool(name="xtp", bufs=2) as xtp, \
                 tc.tile_pool(name="psA", bufs=6, space="PSUM") as psA:

                # first x block queued before the bulky weight loads
                xch0 = []
                for j in range(4):
                    xc = xs.tile([128, D], f32, tag="xc")
                    nc.sync.dma_start(xc[:], X[ts(j, 128), :])
                    xch0.append(xc)

                wqt = sb.tile([128, KC * DL], f32r, tag="wqt")
                nc.sync.dma_start(wqt[:].rearrange("p (c m) -> p c m", c=KC),
                                  WQ.ap().rearrange("(c p) m -> p c m", p=128))
                wkt = sb.tile([128, KC * DL], f32r, tag="wkt")
                nc.sync.dma_start(wkt[:].rearrange("p (c m) -> p c m", c=KC),
                                  WK.ap().rearrange("(c p) m -> p c m", p=128))
                wvt = sb.tile([128, KC * DL], f32r, tag="wvt")
                nc.sync.dma_start(wvt[:].rearrange("p (c m) -> p c m", c=KC),
                                  WV.ap().rearrange("(c p) m -> p c m", p=128))
                bq2 = sb.tile([128, MC], f32, tag="bq2")
                nc.sync.dma_start(bq2[:], BQ[:, :])
                bk2 = sb.tile([128, MC], f32, tag="bk2")
                nc.sync.dma_start(bk2[:], BK[:, :])
                bv_t = sb.tile([1, DL], f32r, tag="bv")
                nc.sync.dma_start(bv_t[:], BV[:, :])
                ones_t = sb.tile([1, 128], f32r, tag="ones")
                nc.sync.dma_start(ones_t[:], ONE[:, :])
                kvm = sb.tile([128, skv], f32, tag="kvm")
                nc.sync.dma_start(kvm[:], KVM[:, :])

                def transpose_block(src_tiles, col0, width, kc, dst, dst_off):
                    """PE-transpose width cols of chunk tiles into dst."""
                    pt = psA.tile([128, 512], f32, tag="sc", name="tp")
                    nw = width // 128
                    for j in range(nw):
                        nc.tensor.transpose(pt[:, ts(j, 128)],
                                            src_tiles[j][:, ts(kc, 128)],
                                            ident[:])
                    return pt

                def drain(dst_ap, src_ap, bias=None, scale=None):
                    nonlocal eng
                    if eng == 0:
                        if bias is not None:
                            nc.vector.tensor_scalar_add(dst_ap, src_ap, bias)
                        elif scale is not None:
                            nc.vector.tensor_scalar_mul(dst_ap, src_ap, scale)
                        else:
                            nc.vector.tensor_copy(dst_ap, src_ap)
                    else:
                        if bias is not None:
                            nc.scalar.add(dst_ap, src_ap, bias)
                        elif scale is not None:
                            nc.scalar.activation(dst_ap, src_ap, AF.Copy,
                                                 scale=scale)
                        else:
                            nc.scalar.copy(dst_ap, src_ap)
                    eng ^= 1

                # ---- A1: Q path over full x ------------------------------
                for bi, (off, w) in enumerate(_blocks(S)):
                    if bi == 0:
                        xch = xch0
                    else:
                        xch = []
                        for j in range(w // 128):
                            xc = xs.tile([128, D], f32, tag="xc")
                            nc.sync.dma_start(xc[:],
                                              X[off + j * 128:off + (j + 1) * 128, :])
                            xch.append(xc)
                    xtb = []
                    for kc in range(KC):
                        pt = transpose_block(xch, off, w, kc, None, None)
                        xb = xtp.tile([128, 512], f32r, tag=f"xt{kc}",
                                      name=f"xt{kc}")
                        drain(xb[:, 0:w], pt[:, 0:w])
                        xtb.append(xb)
                    for mc in range(MC):
                        pt = psA.tile([128, 512], f32, tag="sc", name="qp")
                        for kc in range(KC):
                            nc.tensor.matmul(
                                pt[:, 0:w],
                                wqt[:, kc * DL + mc * 128:kc * DL + mc * 128 + 128],
                                xtb[kc][:, 0:w],
                                start=(kc == 0), stop=(kc == KC - 1))
                        drain(qT[mc][:, off:off + w], pt[:, 0:w],
                              bias=bq2[:, mc:mc + 1])
                    if not compact:
                        # K/V share the same transposed blocks
                        for mc in range(MC):
                            pt = psA.tile([128, 512], f32, tag="sc", name="kp")
                            for kc in range(KC):
                                nc.tensor.matmul(
                                    pt[:, 0:w],
                                    wkt[:, kc * DL + mc * 128:kc * DL + mc * 128 + 128],
                                    xtb[kc][:, 0:w],
                                    start=(kc == 0), stop=(kc == KC - 1))
                            drain(kT[mc][:, off:off + w], pt[:, 0:w],
                                  bias=bk2[:, mc:mc + 1])
                        for j in range(w // 128):
                            sc = (off + j * 128) // 128
                            pv = psA.tile([128, 512], f32, tag="sc", name="vp")
                            nc.tensor.matmul(pv[:, 0:DL], ones_t[:], bv_t[:],
                                             start=True, stop=False)
                            for kc in range(KC):
                                nc.tensor.matmul(pv[:, 0:DL],
                                                 xtb[kc][:, ts(j, 128)],
                                                 wvt[:, ts(kc, DL)],
                                                 start=False,
                                                 stop=(kc == KC - 1))
                            drain(v_sb[:, ts(sc, DL)], pv[:, 0:DL],
                                  scale=kvm[:, sc:sc + 1])

                # ---- A2 (compact): K/V over gathered keys ----------------
                if compact:
                    for off, w in _blocks(CAP):
                        xch = []
                        for j in range(w // 128):
                            xc = xs.tile([128, D], f32, tag="xc")
                            nc.sync.dma_start(
                                xc[:],
                                XKV[off + j * 128:off + (j + 1) * 128, :])
                            xch.append(xc)
                        xtb = []
                        for kc in range(KC):
                            pt = transpose_block(xch, off, w, kc, None, None)
                            xb = xtp.tile([128, 512], f32r, tag=f"xt{kc}",
                                          name=f"xkvt{kc}")
                            drain(xb[:, 0:w], pt[:, 0:w])
                            xtb.append(xb)
                        for mc in range(MC):
                            pt = psA.tile([128, 512], f32, tag="sc", name="kp")
                            for kc in range(KC):
                                nc.tensor.matmul(
                                    pt[:, 0:w],
                                    wkt[:, kc * DL + mc * 128:kc * DL + mc * 128 + 128],
                                    xtb[kc][:, 0:w],
                                    start=(kc == 0), stop=(kc == KC - 1))
                            drain(kT[mc][:, off:off + w], pt[:, 0:w],
                                  bias=bk2[:, mc:mc + 1])
                        for j in range(w // 128):
                            sc = (off + j * 128) // 128
                            pv = psA.tile([128, 512], f32, tag="sc", name="vp")
                            nc.tensor.matmul(pv[:, 0:DL], ones_t[:], bv_t[:],
                                             start=True, stop=False)
                            for kc in range(KC):
                                nc.tensor.matmul(pv[:, 0:DL],
                                                 xtb[kc][:, ts(j, 128)],
                                                 wvt[:, ts(kc, DL)],
                                                 start=False,
                                                 stop=(kc == KC - 1))
                            drain(v_sb[:, ts(sc, DL)], pv[:, 0:DL],
                                  scale=kvm[:, sc:sc + 1])

            # ---- phase B: attention --------------------------------------
            with tc.tile_pool(name="probs", bufs=skv + 4) as pp, \
                 tc.tile_pool(name="psB", bufs=3, space="PSUM") as psB, \
                 tc.tile_pool(name="psc", bufs=2, space="PSUM") as psc:
                for hp in range(MC):
                    for sq in range(NSQ):
                        pbs = []
                        for sk in range(skv):
                            spt = psB.tile([128, 1024], f32, tag="sc2")
                            nc.tensor.matmul(spt[:, 0:512],
                                             kT[hp][0:64, ts(sk, 128)],
                                             qT[hp][0:64, ts(sq, SQW)],
                                             start=True, stop=True)
                            nc.tensor.matmul(spt[:, 512:1024],
                                             kT[hp][64:128, ts(sk, 128)],
                                             qT[hp][64:128, ts(sq, SQW)],
                                             start=True, stop=True)
                            pb = pp.tile([128, 1024], CTX_DT if CTX_BF16 else f32r, tag="pb")
                            if eng == 0:
                                nc.vector.tensor_copy(pb[:], spt[:])
                            else:
                                nc.scalar.copy(pb[:], spt[:])
                            eng ^= 1
                            pbs.append(pb)
                        if CTX_BF16:
                            ct = psc.tile([128, SQW], f32, tag="ctx",
                                          name=f"ct{hp}_{sq}")
                            for sk in range(skv):
                                for h in range(2):
                                    nc.tensor.matmul(
                                        ct[h * 64:(h + 1) * 64, :],
                                        v_sb[:, sk * DL + hp * 128 + h * 64:
                                             sk * DL + hp * 128 + h * 64 + 64],
                                        pbs[sk][:, h * 512:(h + 1) * 512],
                                        start=(sk == 0), stop=(sk == skv - 1),
                                        tile_position=(0, h * 64),
                                        skip_group_check=True)
                            stage = stg.tile([128, SQW], f32, tag="st")
                            if eng == 0:
                                nc.vector.tensor_copy(stage[:], ct[:])
                            else:
                                nc.scalar.copy(stage[:], ct[:])
                            eng ^= 1
                        else:
                            cts = [psc.tile([64, SQW], f32, tag="ctx",
                                            name=f"ct{hp}_{sq}_{i}")
                                   for i in range(2)]
                            for sk in range(skv):
                                for h in range(2):
                                    nc.tensor.matmul(
                                        cts[h][:],
                                        v_sb[:, sk * DL + hp * 128 + h * 64:
                                             sk * DL + hp * 128 + h * 64 + 64],
                                        pbs[sk][:, h * 512:(h + 1) * 512],
                                        start=(sk == 0), stop=(sk == skv - 1))
                            stage = stg.tile([128, SQW], f32, tag="st")
                            nc.vector.tensor_copy(stage[0:64, :], cts[0][:])
                            nc.scalar.copy(stage[64:128, :], cts[1][:])
                        nc.sync.dma_start(
                            OUT[hp * 128:(hp + 1) * 128, ts(sq, SQW)], stage[:])

    nc.compile()
    return nc


def _get_nc(compact):
    key = "compact" if compact else "full"
    if key not in _cache:
        _cache[key] = _build(compact)
    return _cache[key]


def _make_in_maps(hidden_states, attention_mask, Wq, bq, Wk, bk, Wv, bv):
    hs = np.ascontiguousarray(np.asarray(hidden_states, dtype=np.float32))
    am = np.asarray(attention_mask, dtype=np.float32)

    # key compaction metadata per batch
    compact = True
    idxs, kvms, xkvs = [], [], []
    for b in range(B):
        valid = np.nonzero(am[b, 0, 0, :] >= 0)[0]
        if len(valid) > CAP:
            compact = False
            break
        idxp = np.zeros(CAP, np.int64)
        idxp[:len(valid)] = valid
        kvm = np.zeros(CAP, np.float32)
        kvm[:len(valid)] = 1.0
        idxs.append(idxp)
        kvms.append(kvm)
        xkvs.append(np.ascontiguousarray(hs[b][idxp]))

    skv = (CAP if compact else S) // 128
    ones = np.ones((1, 128), np.float32)
    idn = np.eye(128, dtype=np.float32)
    in_maps = []
    for c in range(N_CORES):
        b, g = divmod(c, 4)
        sl = slice(g * DL, (g + 1) * DL)
        if compact:
            kvm2 = np.ascontiguousarray(kvms[b].reshape(skv, 128).T)
        else:
            kvm2 = np.ascontiguousarray(
                (am[b, 0, 0, :] >= 0).astype(np.float32).reshape(skv, 128).T)
        m = {
            "x": hs[b],
            "idn": idn,
            "wqt": np.ascontiguousarray(np.asarray(Wq, np.float32)[sl, :].T),
            "wkt": np.ascontiguousarray(np.asarray(Wk, np.float32)[sl, :].T),
            "wvt": np.ascontiguousarray(np.asarray(Wv, np.float32)[sl, :].T),
            "bq2": np.ascontiguousarray(
                np.asarray(bq, np.float32)[sl].reshape(MC, 128).T),
            "bk2": np.ascontiguousarray(
                np.asarray(bk, np.float32)[sl].reshape(MC, 128).T),
            "bv": np.ascontiguousarray(
                np.asarray(bv, np.float32)[sl].reshape(1, DL)),
            "ones": ones,
            "kvm2": kvm2,
        }
        if compact:
            m["xkv"] = xkvs[b]
        in_maps.append(m)
    return compact, in_maps


def _gather(results):
    out = np.empty((B, S, D), np.float32)
    for c in range(N_CORES):
        b, g = divmod(c, 4)
        out[b, :, g * DL:(g + 1) * DL] = results[c]["out"].T
    return out


def run_sharded(compact, in_maps, **kw):
    nc = _get_nc(compact)
    return run_bass_kernel_spmd(nc, in_maps, core_ids=list(range(N_CORES)), **kw)


def kernel(hidden_states, attention_mask, Wq, bq, Wk, bk, Wv, bv):
    compact, in_maps = _make_in_maps(hidden_states, attention_mask,
                                     Wq, bq, Wk, bk, Wv, bv)
    res = run_sharded(compact, in_maps)
    return _gather(res.results)


# revision 8
# speedup vs baseline: 1.7779x; 1.1342x over previous
"""BertLinearSelfAttention on 8 Trainium2 NeuronCores.

Problem (per reference):
  q = hs @ Wq.T + bq ; k = hs @ Wk.T + bk ; v = hs @ Wv.T + bv   (B,S,D)
  per head: scores = q @ k.T ; probs = scores * (mask >= 0) ; ctx = probs @ v
  B=2, S=2048, D=1024, H=16, HD=64. No softmax, binary key mask.

Sharding: core c = 4*b + g handles batch b and head group g (4 heads,
256 output features). Each core runs the same SPMD program on its own
slice; output is gathered host-side.

Algebraic moves:
  1) (scores * mask_k) @ v == scores @ (mask_k * v): the binary key mask
     applies to V rows instead of the S x S probs matrix.
  2) Masked keys contribute exactly zero, so K/V work only needs the
     valid keys. Inputs are compacted to CAP key slots (valid indices +
     zero-padding); a full-width fallback program handles the
     (astronomically unlikely) case of more than CAP valid keys.

On-chip layout (per core):
  xT blocks [128, 512]  hidden transposed via PE-identity transposes
  qT [256, S], kT [256, CAP] projection outputs kept feature-major
  v  [CAP, 256]  natural layout, bias via K=1 ones-matmul, pad mask
                 applied on the PSUM->SBUF copy
  scoresT pair tiles [s_k 128, 2 x s_q 512] = kT.T @ qT for both heads
                 of a pair (K=64 packed via disjoint PE row groups, two
                 PSUM banks), drained by one wide copy
  ctxT [64, s_q] accumulated over s_k chunks (fp32r, M=64)
All matmuls run as float32r (TF32-like, ~1.5e-4 rel err, 1 cyc/row);
transposes are exact fp32.
"""
import numpy as np
import concourse.bass as bass
import concourse.mybir as mybir
import concourse.tile as tile
from concourse import bacc
from concourse.bass import ts
from concourse.bass_utils import run_bass_kernel_spmd

f32 = mybir.dt.float32
f32r = mybir.dt.float32r
bf16 = mybir.dt.bfloat16
fp16 = mybir.dt.float16
AF = mybir.ActivationFunctionType

B = 2
S = 2048
D = 1024
DL = 256          # output features per core (4 heads x 64)
KC = D // 128     # 8 contraction chunks
SC = S // 128     # 16 sequence chunks
MC = DL // 128    # 2 feature chunks / head pairs
SQW = 512         # attention s_q strip width
NSQ = S // SQW    # 4 strips
N_CORES = 8
CAP = 1152        # compacted key slots (valid count ~Binom(2048,.5), mean
                  # 1024 sd 22.6; 1152 is ~5.7 sigma up, fallback covers more)
CTX_BF16 = True   # probs/v in fp16 -> ctx pairs col-packed in the PE array
CTX_DT = mybir.dt.float16

_cache = {}


def _blocks(width):
    """Split `width` columns into 512-wide blocks (last may be shorter)."""
    out = []
    off = 0
    while off < width:
        w = min(512, width - off)
        out.append((off, w))
        off += w
    return out


def _build(compact):
    skv = (CAP if compact else S) // 128   # key chunks
    nc = bacc.Bacc("TRN2", target_bir_lowering=False, debug=False,
                   num_devices=N_CORES)
    X = nc.declare_dram_parameter("x", [S, D], fp16, isOutput=False)
    if compact:
        XKV = nc.declare_dram_parameter("xkv", [CAP, D], fp16, isOutput=False)
    IDN = nc.declare_dram_parameter("idn", [128, 128], fp16, isOutput=False)
    WQ = nc.declare_dram_parameter("wqt", [D, DL], fp16, isOutput=False)
    WK = nc.declare_dram_parameter("wkt", [D, DL], fp16, isOutput=False)
    WV = nc.declare_dram_parameter("wvt", [D, DL], fp16, isOutput=False)
    BQ = nc.declare_dram_parameter("bq2", [128, MC], f32, isOutput=False)
    BK = nc.declare_dram_parameter("bk2", [128, MC], f32, isOutput=False)
    BV = nc.declare_dram_parameter("bv", [1, DL], fp16, isOutput=False)
    ONE = nc.declare_dram_parameter("ones", [1, 128], fp16, isOutput=False)
    KVM = nc.declare_dram_parameter("kvm2", [128, skv], f32, isOutput=False)
    OUT = nc.declare_dram_parameter("out", [DL, S], f32, isOutput=True)

    with tile.TileContext(nc) as tc:
        with tc.tile_pool(name="sb", bufs=1) as sb, \
             tc.tile_pool(name="stg", bufs=4) as stg:

            ident = sb.tile([128, 128], fp16, tag="ident")
            nc.sync.dma_start(ident[:], IDN[:, :])

            qT = [sb.tile([128, S], f32r, tag=f"qT{m}", name=f"qT{m}")
                  for m in range(MC)]
            kT = [sb.tile([128, CAP if compact else S], f32r,
                          tag=f"kT{m}", name=f"kT{m}") for m in range(MC)]
            v_sb = sb.tile([128, skv * DL], CTX_DT if CTX_BF16 else f32r, tag="v_sb")

            eng = 0  # DVE/ACT alternator for PSUM->SBUF drains

            with tc.tile_pool(name="xs", bufs=6) as xs, \
                 tc.tile_pool(name="xtp", bufs=2) as xtp, \
                 tc.tile_pool(name="psA", bufs=5, space="PSUM") as psA:

                # first x block queued before the bulky weight loads
                xch0 = []
                for j in range(4):
                    xc = xs.tile([128, D], fp16, tag="xc")
                    nc.sync.dma_start(xc[:], X[ts(j, 128), :])
                    xch0.append(xc)

                wqt = sb.tile([128, KC * DL], fp16, tag="wqt")
                nc.sync.dma_start(wqt[:].rearrange("p (c m) -> p c m", c=KC),
                                  WQ.ap().rearrange("(c p) m -> p c m", p=128))
                wkt = sb.tile([128, KC * DL], fp16, tag="wkt")
                nc.sync.dma_start(wkt[:].rearrange("p (c m) -> p c m", c=KC),
                                  WK.ap().rearrange("(c p) m -> p c m", p=128))
                wvt = sb.tile([128, KC * DL], fp16, tag="wvt")
                nc.sync.dma_start(wvt[:].rearrange("p (c m) -> p c m", c=KC),
                                  WV.ap().rearrange("(c p) m -> p c m", p=128))
                bq2 = sb.tile([128, MC], f32, tag="bq2")
                nc.sync.dma_start(bq2[:], BQ[:, :])
                bk2 = sb.tile([128, MC], f32, tag="bk2")
                nc.sync.dma_start(bk2[:], BK[:, :])
                bv_t = sb.tile([1, DL], fp16, tag="bv")
                nc.sync.dma_start(bv_t[:], BV[:, :])
                ones_t = sb.tile([1, 128], fp16, tag="ones")
                nc.sync.dma_start(ones_t[:], ONE[:, :])
                kvm = sb.tile([128, skv], f32, tag="kvm")
                nc.sync.dma_start(kvm[:], KVM[:, :])

                def transpose_block(src_tiles, col0, width, kc, dst, dst_off):
                    """PE-transpose width cols of chunk tiles into dst."""
                    pt = psA.tile([128, 512], fp16, tag="tp", name="tp", bufs=3)
                    nw = width // 128
                    for j in range(nw):
                        nc.tensor.transpose(pt[:, ts(j, 128)],
                                            src_tiles[j][:, ts(kc, 128)],
                                            ident[:])
                    return pt

                def drain(dst_ap, src_ap, bias=None, scale=None):
                    nonlocal eng
                    if eng == 0:
                        if bias is not None:
                            nc.vector.tensor_scalar_add(dst_ap, src_ap, bias)
                        elif scale is not None:
                            nc.vector.tensor_scalar_mul(dst_ap, src_ap, scale)
                        else:
                            nc.vector.tensor_copy(dst_ap, src_ap)
                    else:
                        if bias is not None:
                            nc.scalar.add(dst_ap, src_ap, bias)
                        elif scale is not None:
                            nc.scalar.activation(dst_ap, src_ap, AF.Copy,
                                                 scale=scale)
                        else:
                            nc.scalar.copy(dst_ap, src_ap)
                    eng ^= 1

                # ---- A1: Q path over full x ------------------------------
                for bi, (off, w) in enumerate(_blocks(S)):
                    if bi == 0:
                        xch = xch0
                    else:
                        xch = []
                        for j in range(w // 128):
                            xc = xs.tile([128, D], fp16, tag="xc")
                            nc.sync.dma_start(xc[:],
                                              X[off + j * 128:off + (j + 1) * 128, :])
                            xch.append(xc)
                    xtb = []
                    for kc in range(KC):
                        pt = transpose_block(xch, off, w, kc, None, None)
                        xb = xtp.tile([128, 512], fp16, tag=f"xt{kc}",
                                      name=f"xt{kc}")
                        drain(xb[:, 0:w], pt[:, 0:w])
                        xtb.append(xb)
                    for mc in range(MC):
                        pt = psA.tile([128, 512], f32, tag="sc", name="qp")
                        for kc in range(KC):
                            nc.tensor.matmul(
                                pt[:, 0:w],
                                wqt[:, kc * DL + mc * 128:kc * DL + mc * 128 + 128],
                                xtb[kc][:, 0:w],
                                start=(kc == 0), stop=(kc == KC - 1))
                        drain(qT[mc][:, off:off + w], pt[:, 0:w],
                              bias=bq2[:, mc:mc + 1])
                    if not compact:
                        # K/V share the same transposed blocks
                        for mc in range(MC):
                            pt = psA.tile([128, 512], f32, tag="sc", name="kp")
                            for kc in range(KC):
                                nc.tensor.matmul(
                                    pt[:, 0:w],
                                    wkt[:, kc * DL + mc * 128:kc * DL + mc * 128 + 128],
                                    xtb[kc][:, 0:w],
                                    start=(kc == 0), stop=(kc == KC - 1))
                            drain(kT[mc][:, off:off + w], pt[:, 0:w],
                                  bias=bk2[:, mc:mc + 1])
                        for j in range(w // 128):
                            sc = (off + j * 128) // 128
                            pv = psA.tile([128, 512], f32, tag="sc", name="vp")
                            nc.tensor.matmul(pv[:, 0:DL], ones_t[:], bv_t[:],
                                             start=True, stop=False)
                            for kc in range(KC):
                                nc.tensor.matmul(pv[:, 0:DL],
                                                 xtb[kc][:, ts(j, 128)],
                                                 wvt[:, ts(kc, DL)],
                                                 start=False,
                                                 stop=(kc == KC - 1))
                            drain(v_sb[:, ts(sc, DL)], pv[:, 0:DL],
                                  scale=kvm[:, sc:sc + 1])

                # ---- A2 (compact): K/V over gathered keys ----------------
                if compact:
                    for off, w in _blocks(CAP):
                        xch = []
                        for j in range(w // 128):
                            xc = xs.tile([128, D], fp16, tag="xc")
                            nc.sync.dma_start(
                                xc[:],
                                XKV[off + j * 128:off + (j + 1) * 128, :])
                            xch.append(xc)
                        xtb = []
                        for kc in range(KC):
                            pt = transpose_block(xch, off, w, kc, None, None)
                            xb = xtp.tile([128, 512], fp16, tag=f"xt{kc}",
                                          name=f"xkvt{kc}")
                            drain(xb[:, 0:w], pt[:, 0:w])
                            xtb.append(xb)
                        for mc in range(MC):
                            pt = psA.tile([128, 512], f32, tag="sc", name="kp")
                            for kc in range(KC):
                                nc.tensor.matmul(
                                    pt[:, 0:w],
                                    wkt[:, kc * DL + mc * 128:kc * DL + mc * 128 + 128],
                                    xtb[kc][:, 0:w],
                                    start=(kc == 0), stop=(kc == KC - 1))
                            drain(kT[mc][:, off:off + w], pt[:, 0:w],
                                  bias=bk2[:, mc:mc + 1])
                        for j in range(w // 128):
                            sc = (off + j * 128) // 128
                            pv = psA.tile([128, 512], f32, tag="sc", name="vp")
                            nc.tensor.matmul(pv[:, 0:DL], ones_t[:], bv_t[:],
                                             start=True, stop=False)
                            for kc in range(KC):
                                nc.tensor.matmul(pv[:, 0:DL],
                                                 xtb[kc][:, ts(j, 128)],
                                                 wvt[:, ts(kc, DL)],
                                                 start=False,
                                                 stop=(kc == KC - 1))
                            drain(v_sb[:, ts(sc, DL)], pv[:, 0:DL],
                                  scale=kvm[:, sc:sc + 1])

            # ---- phase B: attention --------------------------------------
            with tc.tile_pool(name="probs", bufs=skv + 4) as pp, \
                 tc.tile_pool(name="psB", bufs=3, space="PSUM") as psB, \
                 tc.tile_pool(name="psc", bufs=2, space="PSUM") as psc:
                for hp in range(MC):
                    for sq in range(NSQ):
                        pbs = []
                        for sk in range(skv):
                            spt = psB.tile([128, 1024], f32, tag="sc2")
                            nc.tensor.matmul(spt[:, 0:512],
                                             kT[hp][0:64, ts(sk, 128)],
                                             qT[hp][0:64, ts(sq, SQW)],
                                             start=True, stop=True)
                            nc.tensor.matmul(spt[:, 512:1024],
                                             kT[hp][64:128, ts(sk, 128)],
                                             qT[hp][64:128, ts(sq, SQW)],
                                             start=True, stop=True)
                            pb = pp.tile([128, 1024], CTX_DT if CTX_BF16 else f32r, tag="pb")
                            if eng == 0:
                                nc.vector.tensor_copy(pb[:], spt[:])
                            else:
                                nc.scalar.copy(pb[:], spt[:])
                            eng ^= 1
                            pbs.append(pb)
                        if CTX_BF16:
                            ct = psc.tile([128, SQW], f32, tag="ctx",
                                          name=f"ct{hp}_{sq}")
                            for sk in range(skv):
                                for h in range(2):
                                    nc.tensor.matmul(
                                        ct[h * 64:(h + 1) * 64, :],
                                        v_sb[:, sk * DL + hp * 128 + h * 64:
                                             sk * DL + hp * 128 + h * 64 + 64],
                                        pbs[sk][:, h * 512:(h + 1) * 512],
                                        start=(sk == 0), stop=(sk == skv - 1),
                                        tile_position=(0, h * 64),
                                        skip_group_check=True)
                            stage = stg.tile([128, SQW], f32, tag="st")
                            if eng == 0:
                                nc.vector.tensor_copy(stage[:], ct[:])
                            else:
                                nc.scalar.copy(stage[:], ct[:])
                            eng ^= 1
                        else:
                            cts = [psc.tile([64, SQW], f32, tag="ctx",
                                            name=f"ct{hp}_{sq}_{i}")
                                   for i in range(2)]
                            for sk in range(skv):
                                for h in range(2):
                                    nc.tensor.matmul(
                                        cts[h][:],
                                        v_sb[:, sk * DL + hp * 128 + h * 64:
                                             sk * DL + hp * 128 + h * 64 + 64],
                                        pbs[sk][:, h * 512:(h + 1) * 512],
                                        start=(sk == 0), stop=(sk == skv - 1))
                            stage = stg.tile([128, SQW], f32, tag="st")
                            nc.vector.tensor_copy(stage[0:64, :], cts[0][:])
                            nc.scalar.copy(stage[64:128, :], cts[1][:])
                        nc.sync.dma_start(
                            OUT[hp * 128:(hp + 1) * 128, ts(sq, SQW)], stage[:])

    nc.compile()
    return nc


def _get_nc(compact):
    key = "compact" if compact else "full"
    if key not in _cache:
        _cache[key] = _build(compact)
    return _cache[key]


def _make_in_maps(hidden_states, attention_mask, Wq, bq, Wk, bk, Wv, bv):
    hs = np.ascontiguousarray(np.asarray(hidden_states, dtype=np.float32))
    hs16 = hs.astype(np.float16)
    am = np.asarray(attention_mask, dtype=np.float32)

    # key compaction metadata per batch
    compact = True
    idxs, kvms, xkvs = [], [], []
    for b in range(B):
        valid = np.nonzero(am[b, 0, 0, :] >= 0)[0]
        if len(valid) > CAP:
            compact = False
            break
        idxp = np.zeros(CAP, np.int64)
        idxp[:len(valid)] = valid
        kvm = np.zeros(CAP, np.float32)
        kvm[:len(valid)] = 1.0
        idxs.append(idxp)
        kvms.append(kvm)
        xkvs.append(np.ascontiguousarray(hs16[b][idxp]))

    skv = (CAP if compact else S) // 128
    ones = np.ones((1, 128), np.float16)
    idn = np.eye(128, dtype=np.float16)
    in_maps = []
    for c in range(N_CORES):
        b, g = divmod(c, 4)
        sl = slice(g * DL, (g + 1) * DL)
        if compact:
            kvm2 = np.ascontiguousarray(kvms[b].reshape(skv, 128).T)
        else:
            kvm2 = np.ascontiguousarray(
                (am[b, 0, 0, :] >= 0).astype(np.float32).reshape(skv, 128).T)
        m = {
            "x": hs16[b],
            "idn": idn,
            "wqt": np.ascontiguousarray(np.asarray(Wq, np.float32)[sl, :].T.astype(np.float16)),
            "wkt": np.ascontiguousarray(np.asarray(Wk, np.float32)[sl, :].T.astype(np.float16)),
            "wvt": np.ascontiguousarray(np.asarray(Wv, np.float32)[sl, :].T.astype(np.float16)),
            "bq2": np.ascontiguousarray(
                np.asarray(bq, np.float32)[sl].reshape(MC, 128).T),
            "bk2": np.ascontiguousarray(
                np.asarray(bk, np.float32)[sl].reshape(MC, 128).T),
            "bv": np.ascontiguousarray(
                np.asarray(bv, np.float32)[sl].reshape(1, DL).astype(np.float16)),
            "ones": ones,
            "kvm2": kvm2,
        }
        if compact:
            m["xkv"] = xkvs[b]
        in_maps.append(m)
    return compact, in_maps


def _gather(results):
    out = np.empty((B, S, D), np.float32)
    for c in range(N_CORES):
        b, g = divmod(c, 4)
        out[b, :, g * DL:(g + 1) * DL] = results[c]["out"].T
    return out


def run_sharded(compact, in_maps, **kw):
    nc = _get_nc(compact)
    return run_bass_kernel_spmd(nc, in_maps, core_ids=list(range(N_CORES)), **kw)


def kernel(hidden_states, attention_mask, Wq, bq, Wk, bk, Wv, bv):
    compact, in_maps = _make_in_maps(hidden_states, attention_mask,
                                     Wq, bq, Wk, bk, Wv, bv)
    res = run_sharded(compact, in_maps)
    return _gather(res.results)
